# revision 19
# baseline (speedup 1.0000x reference)
"""Bahdanau attention kernel for Trainium2 (8 NeuronCores, data-parallel over batch).

Computes, for each batch row b:
    energy  = tanh(enc[b] @ W_e.T + (h[b] @ W_h.T) + b_attn)   # [S, DEC]
    scores  = energy @ v                                        # [S]
    out[b]  = softmax(scores)

Shapes (hardcoded): B=32, S=4096, ENC=512, DEC=512. 8 cores, 4 batch rows/core.

Device-side design (per core):
  - encoder outputs are fed host-transposed as enc_t[b, e, s] (bf16) so the
    contraction dim e lands on SBUF partitions with no on-chip transposes.
  - main matmul: proj[d_chunk(128), s(512)] += W_eT[e_chunk, d_chunk].T @ enc_t[e_chunk, s]
  - ACT fuses bias add (per-partition) + tanh, PSUM -> SBUF (bf16)
  - v-dot: scores[1, s] += v[d_chunk].T @ energy[d_chunk, s]; batch b parked at
    PSUM partition 32*b (its own bank) so ACT exp reads lane-aligned rows.
  - ACT exp with accum_out yields per-partition exp sums; DVE normalizes.
  - Constants ride in two packed params (one f32, one bf16) so each matmul
    carries at most ONE sync wait (walrus Matmult/LDW limit).
"""

import os
import sys

import numpy as np

try:
    import concourse.bass as bass  # noqa: F401
except ImportError:  # toolchain lives in the trn_rl repo
    for p in ("/opt/trn_rl_repo", "/root/.axon_site/_ro/trn_rl_repo"):
        if os.path.isdir(p) and p not in sys.path:
            sys.path.insert(0, p)
    import concourse.bass as bass  # noqa: F401

import ml_dtypes

B, S, ENC, DEC = 32, 4096, 512, 512
N_CORES = 8
BPC = B // N_CORES          # batch rows per core
SG = 512                    # s-columns per matmul group
N_SG = S // SG              # 8 s-groups
KC = ENC // 128             # 4 contraction chunks
DC = DEC // 128             # 4 output-dim chunks

_BF16 = ml_dtypes.bfloat16

_nc_cache = None
last_results = None         # BassKernelResults of the most recent run (for test.py)


def _build_bass():
    import concourse.tile as tile
    from concourse import mybir

    f32 = mybir.dt.float32
    bf16 = mybir.dt.bfloat16
    Act = mybir.ActivationFunctionType

    nc = bass.Bass()

    # whh: [:, k, 0:512] = W_h.T rows k*128..k*128+127 ; [:, k, 512+b] = h[b] chunk k ;
    #      [:, c, 516]   = b_attn chunk c
    # wev: [:, k, 0:512] = W_e.T rows k*128..k*128+127 ; [:, c, 512] = v chunk c
    enc_t = nc.declare_dram_parameter("enc_t", [BPC, ENC, S], bf16, isOutput=False)
    wev_d = nc.declare_dram_parameter("wev", [128, KC, DEC + 1], bf16, isOutput=False)
    whh_d = nc.declare_dram_parameter("whh", [128, KC, DEC + BPC + 1], f32, isOutput=False)
    out_d = nc.declare_dram_parameter("out", [BPC, S], f32, isOutput=True)

    with tile.TileContext(nc) as tc:
        with (
            tc.tile_pool(name="consts", bufs=1) as consts,
            tc.tile_pool(name="encp", bufs=32) as encp,
            tc.tile_pool(name="enp", bufs=3) as enp,
            tc.tile_pool(name="psp", bufs=7, space="PSUM") as psp,
            tc.tile_pool(name="decp", bufs=1, space="PSUM") as decp,
            tc.tile_pool(name="smp", bufs=1) as smp,
        ):
            wev = consts.tile([128, KC, DEC + 1], bf16)
            nc.sync.dma_start(out=wev[:], in_=wev_d[:, :, :])
            whh = consts.tile([128, KC, DEC + BPC + 1], f32)
            nc.sync.dma_start(out=whh[:], in_=whh_d[:, :, :])

            expd = smp.tile([128, S], f32)
            nc.vector.memset(expd[:, :], 0.0)
            sums8 = smp.tile([128, N_SG], f32)
            nc.vector.memset(sums8[:, :], 1.0)

            # decoder projection bias: bias_sb[:, b*DC+c] = (W_h @ h_b + b_attn)[c*128:...]
            psd = decp.tile([128, 32], f32)
            for b in range(BPC):
                for c in range(DC):
                    col = b * DC + c
                    for k in range(KC):
                        nc.tensor.matmul(
                            psd[:, col : col + 1],
                            whh[:, k, c * 128 : (c + 1) * 128],
                            whh[:, k, DEC + b : DEC + b + 1],
                            start=(k == 0),
                            stop=(k == KC - 1),
                        )
            # absorb the wev DMA wait into the PE's vector clock (walrus allows
            # only one sync wait per Matmult; later matmuls must only wait on
            # their enc DMA)
            nc.tensor.matmul(
                psd[:, 16:17],
                wev[:, 0, 0:128],
                wev[:, 0, DEC : DEC + 1],
                start=True,
                stop=True,
            )

            # absorb the whh DMA wait into DVE's vector clock before the bias adds
            dve_scratch = consts.tile([128, 1], f32)
            nc.vector.tensor_copy(dve_scratch[:, :], whh[:, 0, 0:1])

            bias_sb = consts.tile([128, DC * BPC], f32)
            for b in range(BPC):
                nc.vector.tensor_add(
                    out=bias_sb[:, b * DC : (b + 1) * DC],
                    in0=psd[:, b * DC : (b + 1) * DC],
                    in1=whh[:, :, DEC + BPC],
                )
            # Re-materialize the bias through ACT so every tanh's bias dep is a
            # same-queue (ACT) edge instead of a second foreign sync wait --
            # walrus allows one foreign sync wait per datapath instruction.
            bias_act = consts.tile([128, DC * BPC], f32)
            nc.scalar.copy(bias_act[:, :], bias_sb[:, :])
            # dummy activation: takes the one-time ACT table-load pseudo-inst
            # (and its sync waits) instead of the first real tanh
            act_warm = consts.tile([128, 1], f32)
            nc.scalar.activation(
                act_warm[:, :], bias_act[:, 0:1], func=Act.Tanh
            )
            # give ACT an early observation of the PE prelude (dec matmuls +
            # wev absorber) so later ACT ops need no extra PE wait
            act_warm2 = consts.tile([128, 1], f32)
            nc.scalar.copy(act_warm2[:, :], psd[:, 16:17])

            for sg in range(N_SG):
                for b in range(BPC):
                    enc_tile = encp.tile([128, KC, SG], bf16)
                    nc.sync.dma_start(
                        out=enc_tile[:],
                        in_=enc_t[b, :, sg * SG : (sg + 1) * SG].rearrange(
                            "(k p) s -> p k s", p=128
                        ),
                    )
                    # spare last column: the "claim" write below must not overlap
                    # the tanh outputs (that WAW would cost a second sync wait)
                    en_tile = enp.tile([128, DC, SG + 1], bf16)
                    # "claim" the recycled energy slot with a trivial ACT write:
                    # it carries the slot-release wait alone, so the tanh below
                    # only needs its single PE (matmul) wait.
                    nc.scalar.copy(en_tile[:, 0, SG : SG + 1], bias_act[:, 0:1])
                    scores_ps = psp.tile(
                        [128, SG], f32, tag="scores", name="scores", bufs=2
                    )
                    for c in range(DC):
                        pp = psp.tile([128, SG], f32, tag="proj", name="proj", bufs=5)
                        for k in range(KC):
                            nc.tensor.matmul(
                                pp[:, :],
                                wev[:, k, c * 128 : (c + 1) * 128],
                                enc_tile[:, k, :],
                                start=(k == 0),
                                stop=(k == KC - 1),
                            )
                        nc.scalar.activation(
                            out=en_tile[:, c, 0:SG],
                            in_=pp[:, :],
                            func=Act.Tanh,
                            bias=bias_act[:, b * DC + c : b * DC + c + 1],
                        )
                        nc.tensor.matmul(
                            scores_ps[32 * b : 32 * b + 1, :],
                            wev[:, c, DEC : DEC + 1],
                            en_tile[:, c, 0:SG],
                            start=(c == 0),
                            stop=(c == DC - 1),
                            tile_position=(0, 32 * b),
                        )
                    nc.scalar.activation(
                        out=expd[32 * b : 32 * b + 1, sg * SG : (sg + 1) * SG],
                        in_=scores_ps[32 * b : 32 * b + 1, :],
                        func=Act.Exp,
                        accum_out=sums8[32 * b : 32 * b + 1, sg : sg + 1],
                    )

            sums = smp.tile([128, 1], f32)
            nc.vector.reduce_sum(sums[:, :], sums8[:, :], axis=mybir.AxisListType.X)
            recip = smp.tile([128, 1], f32)
            nc.vector.reciprocal(recip[:, :], sums[:, :])
            # write the normalized result to a DVE-owned tile so the out DMA
            # has a single (DVE) dependency
            out_sb = smp.tile([128, S], f32)
            nc.vector.tensor_scalar_mul(
                out=out_sb[:, :], in0=expd[:, :], scalar1=recip[:, :]
            )
            # SWDGE: its DMA lanes are otherwise unused, so no lane-reuse wait
            nc.gpsimd.dma_start(out=out_d[:, :], in_=out_sb[0:128:32, :])

    _split_multi_waits(nc)
    return nc


def _split_multi_waits(nc):
    """This walrus build allows ONE sync wait per instruction. The kernel body
    is engineered to respect that; Tile's auto-emitted tail drain is not (it
    waits on every processor). Split any multi-wait instruction into a chain
    of single-wait drains on the same engine followed by the original."""
    from concourse import mybir

    for bb in nc.main_func.blocks:
        new_insts = []
        for ins in bb.instructions:
            si = getattr(ins, "sync_info", None)
            if si is not None and si.on_wait and len(si.on_wait) > 1:
                waits = list(si.on_wait)
                for w in waits[:-1]:
                    d = mybir.InstDrain(
                        name=nc.get_next_instruction_name(),
                        ins=[],
                        outs=[],
                        bass_is_fusable=False,
                    )
                    d.engine = ins.engine
                    d.sync_info = mybir.SyncInfo(on_wait=[w], on_update=[])
                    nc.register_instruction(d)
                    new_insts.append(d)
                si.on_wait = waits[-1:]
            new_insts.append(ins)
        bb.instructions[:] = new_insts


def _get_nc():
    global _nc_cache
    if _nc_cache is None:
        _nc_cache = _build_bass()
    return _nc_cache


def _prep_in_maps(decoder_hidden, encoder_outputs, W_attn, b_attn, v):
    decoder_hidden = np.asarray(decoder_hidden, dtype=np.float32)
    encoder_outputs = np.asarray(encoder_outputs, dtype=np.float32)
    W_attn = np.asarray(W_attn, dtype=np.float32)
    b_attn = np.asarray(b_attn, dtype=np.float32)
    v = np.asarray(v, dtype=np.float32)

    W_h = W_attn[:, :DEC]           # [d_out, d_in]
    W_e = W_attn[:, DEC:]           # [d_out, e]

    wev = np.zeros((128, KC, DEC + 1), dtype=_BF16)
    wev[:, :, :DEC] = (
        W_e.T.astype(_BF16).reshape(KC, 128, DEC).transpose(1, 0, 2)
    )
    wev[:, :, DEC] = v.astype(_BF16).reshape(DC, 128).T

    whh_base = np.zeros((128, KC, DEC + BPC + 1), dtype=np.float32)
    whh_base[:, :, :DEC] = W_h.T.reshape(KC, 128, DEC).transpose(1, 0, 2)
    whh_base[:, :, DEC + BPC] = b_attn.reshape(DC, 128).T

    # [B, S, E] -> per-core [BPC, E, S] in bf16
    enc_bt = encoder_outputs.transpose(0, 2, 1).astype(_BF16)

    in_maps = []
    for core in range(N_CORES):
        sl = slice(core * BPC, (core + 1) * BPC)
        h = decoder_hidden[sl]                                   # [BPC, 512]
        whh = whh_base.copy()
        whh[:, :, DEC : DEC + BPC] = h.reshape(BPC, KC, 128).transpose(2, 1, 0)
        in_maps.append(
            {
                "enc_t": np.ascontiguousarray(enc_bt[sl]),
                "wev": wev,
                "whh": whh,
            }
        )
    return in_maps


def _ensure_ntff_hook():
    """The agent image's ``antenv`` lacks ``axon_hooks``; synthesize it with a
    ctypes-based NTFF profile hook against the injected libaxon (trace runs only)."""
    try:
        from antenv.axon_hooks import get_axon_ntff_profile_hook  # noqa: F401

        return
    except ImportError:
        pass

    import contextlib
    import ctypes
    import types

    so_path = "/opt/axon/libaxon_pjrt.so"
    hook = None
    if os.path.exists(so_path):
        lib = ctypes.CDLL(so_path)
        if hasattr(lib, "axon_start_nrt_profile"):
            lib.axon_start_nrt_profile.argtypes = [
                ctypes.POINTER(ctypes.c_int64),
                ctypes.c_size_t,
            ]
            lib.axon_start_nrt_profile.restype = ctypes.c_int64
            lib.axon_stop_nrt_profile.argtypes = [ctypes.c_char_p]
            lib.axon_stop_nrt_profile.restype = ctypes.c_int64

            @contextlib.contextmanager
            def _hook(output_dir, device_ids):
                import jax

                jax.devices()
                if device_ids:
                    ids = (ctypes.c_int64 * len(device_ids))(*device_ids)
                    rc = lib.axon_start_nrt_profile(ids, len(device_ids))
                else:
                    rc = lib.axon_start_nrt_profile(None, 0)
                if rc != 0:
                    raise RuntimeError(f"axon_start_nrt_profile rc={rc}")
                try:
                    yield
                finally:
                    n = lib.axon_stop_nrt_profile(str(output_dir).encode())
                    if n <= 0:
                        print(f"ntff capture wrote {n} files", file=sys.stderr)

            hook = _hook

    holder = {"h": hook}
    mod = types.ModuleType("antenv.axon_hooks")
    mod.get_axon_ntff_profile_hook = lambda: holder["h"]
    mod.set_axon_ntff_profile_hook = lambda h: holder.__setitem__("h", h)
    sys.modules["antenv.axon_hooks"] = mod
    import antenv

    antenv.axon_hooks = mod


def kernel(decoder_hidden, encoder_outputs, W_attn, b_attn, v):
    global last_results
    import concourse.bass_utils as bass_utils
    from concourse.bass_utils import run_bass_kernel_spmd

    nc = _get_nc()
    in_maps = _prep_in_maps(decoder_hidden, encoder_outputs, W_attn, b_attn, v)

    trace = os.environ.get("BAHDANAU_TRACE", "0") == "1"
    kwargs = {}
    if trace:
        _ensure_ntff_hook()
        bass_utils.upload_artifacts = lambda tmpdir: str(tmpdir)  # no bucket here
        kwargs["trace"] = True
        tmpdir = os.environ.get("BAHDANAU_TRACE_DIR")
        if tmpdir:
            os.makedirs(tmpdir, exist_ok=True)
            kwargs["tmpdir"] = tmpdir

    res = run_bass_kernel_spmd(nc, in_maps, core_ids=list(range(N_CORES)), **kwargs)
    last_results = res
    out = np.concatenate([res.results[c]["out"] for c in range(N_CORES)], axis=0)
    return out.astype(np.float32)


# revision 20
# speedup vs baseline: 1.0016x; 1.0016x over previous
"""Bahdanau attention kernel for Trainium2 (8 NeuronCores, data-parallel over batch).

Computes, for each batch row b:
    energy  = tanh(enc[b] @ W_e.T + (h[b] @ W_h.T) + b_attn)   # [S, DEC]
    scores  = energy @ v                                        # [S]
    out[b]  = softmax(scores)

Shapes (hardcoded): B=32, S=4096, ENC=512, DEC=512. 8 cores, 4 batch rows/core.

Device-side design (per core):
  - encoder outputs are fed host-transposed as enc_t[b, e, s] (bf16) so the
    contraction dim e lands on SBUF partitions with no on-chip transposes.
  - main matmul: proj[d_chunk(128), s(512)] += W_eT[e_chunk, d_chunk].T @ enc_t[e_chunk, s]
  - ACT fuses bias add (per-partition) + tanh, PSUM -> SBUF (bf16)
  - v-dot: scores[1, s] += v[d_chunk].T @ energy[d_chunk, s]; batch b parked at
    PSUM partition 32*b (its own bank) so ACT exp reads lane-aligned rows.
  - ACT exp with accum_out yields per-partition exp sums; DVE normalizes.
  - Constants ride in two packed params (one f32, one bf16) so each matmul
    carries at most ONE sync wait (walrus Matmult/LDW limit).
"""

import os
import sys

import numpy as np

try:
    import concourse.bass as bass  # noqa: F401
except ImportError:  # toolchain lives in the trn_rl repo
    for p in ("/opt/trn_rl_repo", "/root/.axon_site/_ro/trn_rl_repo"):
        if os.path.isdir(p) and p not in sys.path:
            sys.path.insert(0, p)
    import concourse.bass as bass  # noqa: F401

import ml_dtypes

B, S, ENC, DEC = 32, 4096, 512, 512
N_CORES = 8
BPC = B // N_CORES          # batch rows per core
SG = 512                    # s-columns per matmul group
N_SG = S // SG              # 8 s-groups
KC = ENC // 128             # 4 contraction chunks
DC = DEC // 128             # 4 output-dim chunks

_BF16 = ml_dtypes.bfloat16

_nc_cache = None
last_results = None         # BassKernelResults of the most recent run (for test.py)


def _build_bass():
    import concourse.tile as tile
    from concourse import mybir

    f32 = mybir.dt.float32
    bf16 = mybir.dt.bfloat16
    Act = mybir.ActivationFunctionType

    nc = bass.Bass()

    # whh: [:, k, 0:512] = W_h.T rows k*128..k*128+127 ; [:, k, 512+b] = h[b] chunk k ;
    #      [:, c, 516]   = b_attn chunk c
    # wev: [:, k, 0:512] = W_e.T rows k*128..k*128+127 ; [:, c, 512] = v chunk c
    enc_t = nc.declare_dram_parameter("enc_t", [BPC, ENC, S], bf16, isOutput=False)
    wev_d = nc.declare_dram_parameter("wev", [128, KC, DEC + 1], bf16, isOutput=False)
    whh_d = nc.declare_dram_parameter("whh", [128, KC, DEC + BPC + 1], f32, isOutput=False)
    out_d = nc.declare_dram_parameter("out", [BPC, S], f32, isOutput=True)

    with tile.TileContext(nc) as tc:
        with (
            tc.tile_pool(name="consts", bufs=1) as consts,
            tc.tile_pool(name="encp", bufs=32) as encp,
            tc.tile_pool(name="enp", bufs=3) as enp,
            tc.tile_pool(name="psp", bufs=7, space="PSUM") as psp,
            tc.tile_pool(name="decp", bufs=1, space="PSUM") as decp,
            tc.tile_pool(name="smp", bufs=1) as smp,
        ):
            wev = consts.tile([128, KC, DEC + 1], bf16)
            nc.sync.dma_start(out=wev[:], in_=wev_d[:, :, :])
            whh = consts.tile([128, KC, DEC + BPC + 1], f32)
            nc.sync.dma_start(out=whh[:], in_=whh_d[:, :, :])

            expd = smp.tile([128, S], f32)
            nc.vector.memset(expd[:, :], 0.0)
            sums8 = smp.tile([128, N_SG], f32)
            nc.vector.memset(sums8[:, :], 1.0)

            # decoder projection bias: bias_sb[:, b*DC+c] = (W_h @ h_b + b_attn)[c*128:...]
            psd = decp.tile([128, 32], f32)
            for b in range(BPC):
                for c in range(DC):
                    col = b * DC + c
                    for k in range(KC):
                        nc.tensor.matmul(
                            psd[:, col : col + 1],
                            whh[:, k, c * 128 : (c + 1) * 128],
                            whh[:, k, DEC + b : DEC + b + 1],
                            start=(k == 0),
                            stop=(k == KC - 1),
                        )
            # absorb the wev DMA wait into the PE's vector clock (walrus allows
            # only one sync wait per Matmult; later matmuls must only wait on
            # their enc DMA)
            nc.tensor.matmul(
                psd[:, 16:17],
                wev[:, 0, 0:128],
                wev[:, 0, DEC : DEC + 1],
                start=True,
                stop=True,
            )

            # absorb the whh DMA wait into DVE's vector clock before the bias adds
            dve_scratch = consts.tile([128, 1], f32)
            nc.vector.tensor_copy(dve_scratch[:, :], whh[:, 0, 0:1])

            bias_sb = consts.tile([128, DC * BPC], f32)
            for b in range(BPC):
                nc.vector.tensor_add(
                    out=bias_sb[:, b * DC : (b + 1) * DC],
                    in0=psd[:, b * DC : (b + 1) * DC],
                    in1=whh[:, :, DEC + BPC],
                )
            # Re-materialize the bias through ACT so every tanh's bias dep is a
            # same-queue (ACT) edge instead of a second foreign sync wait --
            # walrus allows one foreign sync wait per datapath instruction.
            bias_act = consts.tile([128, DC * BPC], f32)
            nc.scalar.copy(bias_act[:, :], bias_sb[:, :])
            # dummy activation: takes the one-time ACT table-load pseudo-inst
            # (and its sync waits) instead of the first real tanh
            act_warm = consts.tile([128, 1], f32)
            nc.scalar.activation(
                act_warm[:, :], bias_act[:, 0:1], func=Act.Tanh
            )
            # give ACT an early observation of the PE prelude (dec matmuls +
            # wev absorber) so later ACT ops need no extra PE wait
            act_warm2 = consts.tile([128, 1], f32)
            nc.scalar.copy(act_warm2[:, :], psd[:, 16:17])

            for sg in range(N_SG):
                for b in range(BPC):
                    enc_tile = encp.tile([128, KC, SG], bf16)
                    nc.sync.dma_start(
                        out=enc_tile[:],
                        in_=enc_t[b, :, sg * SG : (sg + 1) * SG].rearrange(
                            "(k p) s -> p k s", p=128
                        ),
                    )
                    # spare last column: the "claim" write below must not overlap
                    # the tanh outputs (that WAW would cost a second sync wait)
                    en_tile = enp.tile([128, DC, SG + 1], bf16)
                    # "claim" the recycled energy slot with a trivial ACT write:
                    # it carries the slot-release wait alone, so the tanh below
                    # only needs its single PE (matmul) wait.
                    nc.scalar.copy(en_tile[:, 0, SG : SG + 1], bias_act[:, 0:1])
                    scores_ps = psp.tile(
                        [128, SG], f32, tag="scores", name="scores", bufs=2
                    )
                    for c in range(DC):
                        pp = psp.tile([128, SG], f32, tag="proj", name="proj", bufs=5)
                        for k in range(KC):
                            nc.tensor.matmul(
                                pp[:, :],
                                wev[:, k, c * 128 : (c + 1) * 128],
                                enc_tile[:, k, :],
                                start=(k == 0),
                                stop=(k == KC - 1),
                            )
                        nc.scalar.activation(
                            out=en_tile[:, c, 0:SG],
                            in_=pp[:, :],
                            func=Act.Tanh,
                            bias=bias_act[:, b * DC + c : b * DC + c + 1],
                        )
                        nc.tensor.matmul(
                            scores_ps[32 * b : 32 * b + 1, :],
                            wev[:, c, DEC : DEC + 1],
                            en_tile[:, c, 0:SG],
                            start=(c == 0),
                            stop=(c == DC - 1),
                            tile_position=(0, 32 * b),
                        )
                    nc.scalar.activation(
                        out=expd[32 * b : 32 * b + 1, sg * SG : (sg + 1) * SG],
                        in_=scores_ps[32 * b : 32 * b + 1, :],
                        func=Act.Exp,
                        accum_out=sums8[32 * b : 32 * b + 1, sg : sg + 1],
                    )

            sums = smp.tile([128, 1], f32)
            nc.vector.reduce_sum(sums[:, :], sums8[:, :], axis=mybir.AxisListType.X)
            recip = smp.tile([128, 1], f32)
            nc.vector.reciprocal(recip[:, :], sums[:, :])
            # write the normalized result to a DVE-owned tile so the out DMA
            # has a single (DVE) dependency
            out_sb = smp.tile([128, S], f32)
            nc.vector.tensor_scalar_mul(
                out=out_sb[:, :], in0=expd[:, :], scalar1=recip[:, :]
            )
            # SWDGE: its DMA lanes are otherwise unused, so no lane-reuse wait
            nc.gpsimd.dma_start(out=out_d[:, :], in_=out_sb[0:128:32, :])

    _split_multi_waits(nc)
    return nc


def _split_multi_waits(nc):
    """This walrus build allows ONE sync wait per instruction. The kernel body
    is engineered to respect that; Tile's auto-emitted tail drain is not (it
    waits on every processor). Split any multi-wait instruction into a chain
    of single-wait drains on the same engine followed by the original."""
    from concourse import mybir

    for bb in nc.main_func.blocks:
        new_insts = []
        for ins in bb.instructions:
            si = getattr(ins, "sync_info", None)
            if si is not None and si.on_wait and len(si.on_wait) > 1:
                waits = list(si.on_wait)
                for w in waits[:-1]:
                    d = mybir.InstDrain(
                        name=nc.get_next_instruction_name(),
                        ins=[],
                        outs=[],
                        bass_is_fusable=False,
                    )
                    d.engine = ins.engine
                    d.sync_info = mybir.SyncInfo(on_wait=[w], on_update=[])
                    nc.register_instruction(d)
                    new_insts.append(d)
                si.on_wait = waits[-1:]
            new_insts.append(ins)
        bb.instructions[:] = new_insts


def _get_nc():
    global _nc_cache
    if _nc_cache is None:
        _nc_cache = _build_bass()
    return _nc_cache


def _prep_in_maps(decoder_hidden, encoder_outputs, W_attn, b_attn, v):
    decoder_hidden = np.asarray(decoder_hidden, dtype=np.float32)
    encoder_outputs = np.asarray(encoder_outputs, dtype=np.float32)
    W_attn = np.asarray(W_attn, dtype=np.float32)
    b_attn = np.asarray(b_attn, dtype=np.float32)
    v = np.asarray(v, dtype=np.float32)

    W_h = W_attn[:, :DEC]           # [d_out, d_in]
    W_e = W_attn[:, DEC:]           # [d_out, e]

    wev = np.zeros((128, KC, DEC + 1), dtype=_BF16)
    wev[:, :, :DEC] = (
        W_e.T.astype(_BF16).reshape(KC, 128, DEC).transpose(1, 0, 2)
    )
    wev[:, :, DEC] = v.astype(_BF16).reshape(DC, 128).T

    whh_base = np.zeros((128, KC, DEC + BPC + 1), dtype=np.float32)
    whh_base[:, :, :DEC] = W_h.T.reshape(KC, 128, DEC).transpose(1, 0, 2)
    whh_base[:, :, DEC + BPC] = b_attn.reshape(DC, 128).T

    # [B, S, E] -> per-core [BPC, E, S] in bf16
    enc_bt = encoder_outputs.transpose(0, 2, 1).astype(_BF16)

    in_maps = []
    for core in range(N_CORES):
        sl = slice(core * BPC, (core + 1) * BPC)
        h = decoder_hidden[sl]                                   # [BPC, 512]
        whh = whh_base.copy()
        whh[:, :, DEC : DEC + BPC] = h.reshape(BPC, KC, 128).transpose(2, 1, 0)
        in_maps.append(
            {
                "enc_t": np.ascontiguousarray(enc_bt[sl]),
                "wev": wev,
                "whh": whh,
            }
        )
    return in_maps


def _ensure_ntff_hook():
    """The agent image's ``antenv`` lacks ``axon_hooks``; synthesize it with a
    ctypes-based NTFF profile hook against the injected libaxon (trace runs only)."""
    try:
        from antenv.axon_hooks import get_axon_ntff_profile_hook  # noqa: F401

        return
    except ImportError:
        pass

    import contextlib
    import ctypes
    import types

    so_path = "/opt/axon/libaxon_pjrt.so"
    hook = None
    if os.path.exists(so_path):
        lib = ctypes.CDLL(so_path)
        if hasattr(lib, "axon_start_nrt_profile"):
            lib.axon_start_nrt_profile.argtypes = [
                ctypes.POINTER(ctypes.c_int64),
                ctypes.c_size_t,
            ]
            lib.axon_start_nrt_profile.restype = ctypes.c_int64
            lib.axon_stop_nrt_profile.argtypes = [ctypes.c_char_p]
            lib.axon_stop_nrt_profile.restype = ctypes.c_int64

            @contextlib.contextmanager
            def _hook(output_dir, device_ids):
                import jax

                jax.devices()
                if device_ids:
                    ids = (ctypes.c_int64 * len(device_ids))(*device_ids)
                    rc = lib.axon_start_nrt_profile(ids, len(device_ids))
                else:
                    rc = lib.axon_start_nrt_profile(None, 0)
                if rc != 0:
                    raise RuntimeError(f"axon_start_nrt_profile rc={rc}")
                try:
                    yield
                finally:
                    n = lib.axon_stop_nrt_profile(str(output_dir).encode())
                    if n <= 0:
                        print(f"ntff capture wrote {n} files", file=sys.stderr)

            hook = _hook

    holder = {"h": hook}
    mod = types.ModuleType("antenv.axon_hooks")
    mod.get_axon_ntff_profile_hook = lambda: holder["h"]
    mod.set_axon_ntff_profile_hook = lambda h: holder.__setitem__("h", h)
    sys.modules["antenv.axon_hooks"] = mod
    import antenv

    antenv.axon_hooks = mod


def kernel(decoder_hidden, encoder_outputs, W_attn, b_attn, v):
    global last_results
    import concourse.bass_utils as bass_utils
    from concourse.bass_utils import run_bass_kernel_spmd

    nc = _get_nc()
    in_maps = _prep_in_maps(decoder_hidden, encoder_outputs, W_attn, b_attn, v)

    trace = os.environ.get("BAHDANAU_TRACE", "0") == "1"
    kwargs = {}
    if trace:
        _ensure_ntff_hook()
        bass_utils.upload_artifacts = lambda tmpdir: str(tmpdir)  # no bucket here
        kwargs["trace"] = True
        tmpdir = os.environ.get("BAHDANAU_TRACE_DIR")
        if tmpdir:
            import uuid

            tmpdir = os.path.join(tmpdir, uuid.uuid4().hex[:8])
            os.makedirs(tmpdir, exist_ok=True)
            kwargs["tmpdir"] = tmpdir

    res = run_bass_kernel_spmd(nc, in_maps, core_ids=list(range(N_CORES)), **kwargs)
    last_results = res
    out = np.concatenate([res.results[c]["out"] for c in range(N_CORES)], axis=0)
    return out.astype(np.float32)


# revision 23
# speedup vs baseline: 1.1289x; 1.1271x over previous
"""Bahdanau attention kernel for Trainium2 (8 NeuronCores, data-parallel over batch).

Computes, for each batch row b:
    energy  = tanh(enc[b] @ W_e.T + (h[b] @ W_h.T) + b_attn)   # [S, DEC]
    scores  = energy @ v                                        # [S]
    out[b]  = softmax(scores)

Shapes (hardcoded): B=32, S=4096, ENC=512, DEC=512. 8 cores, 4 batch rows/core.

Device-side design (per core):
  - encoder outputs are fed host-pre-tiled as [b, sg_pair, p, k, s] (bf16) so
    the contraction dim e lands on SBUF partitions with no on-chip transposes
    and each DMA reads 8KB contiguous per partition.
  - main matmul: proj[d_chunk(128), s(512)] += W_eT[e_chunk, d_chunk].T @ enc[e_chunk, s]
  - ACT fuses bias add (per-partition) + tanh over a 2-bank [128,1024] PSUM pair
  - v-dot: scores[1, s] += v[d_chunk].T @ energy[d_chunk, s]; batch b parked at
    PSUM partition 32*b of its own scores bank.
  - ACT exp with accum_out yields per-partition exp sums; DVE normalizes.
  - All constants ride in ONE packed bf16 param; this walrus build allows one
    sync wait per instruction, so the dataflow is engineered for single-wait
    instructions and a post-pass splits any leftovers into wait-only drains.
"""

import os
import sys

import numpy as np

try:
    import concourse.bass as bass  # noqa: F401
except ImportError:  # toolchain lives in the trn_rl repo
    for p in ("/opt/trn_rl_repo", "/root/.axon_site/_ro/trn_rl_repo"):
        if os.path.isdir(p) and p not in sys.path:
            sys.path.insert(0, p)
    import concourse.bass as bass  # noqa: F401

import ml_dtypes

B, S, ENC, DEC = 32, 4096, 512, 512
N_CORES = 8
BPC = B // N_CORES          # batch rows per core
SG = 512                    # s-columns per matmul group
SG2 = 2 * SG                # s-columns per DMA tile / tanh
N_PR = S // SG2             # 4 s-group pairs
KC = ENC // 128             # 4 contraction chunks
DC = DEC // 128             # 4 output-dim chunks

# packed constant layout (bf16): [128, KC, NPK]
_WE0 = 0            # W_e.T            cols [0, 512)
_WH0 = DEC          # W_h.T            cols [512, 1024)
_V0 = 2 * DEC       # v                col  1024
_H0 = 2 * DEC + 1   # decoder hidden   cols [1025, 1025+BPC)
_B0 = _H0 + BPC     # b_attn           col  1029
NPK = _B0 + 1

_BF16 = ml_dtypes.bfloat16

_nc_cache = None
last_results = None         # BassKernelResults of the most recent run (for test.py)


def _build_bass():
    import concourse.tile as tile
    from concourse import mybir

    f32 = mybir.dt.float32
    bf16 = mybir.dt.bfloat16
    Act = mybir.ActivationFunctionType

    nc = bass.Bass()

    enc_d = nc.declare_dram_parameter(
        "enc_t", [BPC, N_PR, 128, KC, SG2], bf16, isOutput=False
    )
    pk_d = nc.declare_dram_parameter("pk", [128, KC, NPK], bf16, isOutput=False)
    out_d = nc.declare_dram_parameter("out", [BPC, S], f32, isOutput=True)

    with tile.TileContext(nc) as tc:
        with (
            tc.tile_pool(name="consts", bufs=1) as consts,
            tc.tile_pool(name="encp", bufs=BPC * N_PR) as encp,
            tc.tile_pool(name="enp", bufs=3) as enp,
            tc.tile_pool(name="psp", bufs=2, space="PSUM") as psp,
            tc.tile_pool(name="decp", bufs=1, space="PSUM") as decp,
            tc.tile_pool(name="smp", bufs=1) as smp,
        ):
            pk = consts.tile([128, KC, NPK], bf16)
            nc.sync.dma_start(out=pk[:], in_=pk_d[:, :, :])

            expd = smp.tile([128, S], f32)
            nc.vector.memset(expd[:, :], 0.0)
            sums8 = smp.tile([128, 2 * N_PR], f32)
            nc.vector.memset(sums8[:, :], 1.0)

            # decoder projection: psd[:, c*BPC+b] = (W_h @ h_b)[c*128:(c+1)*128]
            # batched over the 4 batch rows (N=4 matmuls)
            psd = decp.tile([128, DC * BPC], f32)
            for c in range(DC):
                for k in range(KC):
                    nc.tensor.matmul(
                        psd[:, c * BPC : (c + 1) * BPC],
                        pk[:, k, _WH0 + c * 128 : _WH0 + (c + 1) * 128],
                        pk[:, k, _H0 : _H0 + BPC],
                        start=(k == 0),
                        stop=(k == KC - 1),
                    )

            # f32 copy of b_attn columns; also absorbs the pk DMA into DVE's clock
            b_cols = consts.tile([128, DC], f32)
            nc.vector.tensor_copy(b_cols[:, :], pk[:, :, _B0])
            bias_sb = consts.tile([128, DC * BPC], f32)
            for c in range(DC):
                nc.vector.tensor_scalar_add(
                    out=bias_sb[:, c * BPC : (c + 1) * BPC],
                    in0=psd[:, c * BPC : (c + 1) * BPC],
                    scalar1=b_cols[:, c : c + 1],
                )
            # re-materialize the bias through ACT: tanh's bias dep becomes a
            # same-queue edge (single-sync-wait constraint)
            bias_act = consts.tile([128, DC * BPC], f32)
            nc.scalar.copy(bias_act[:, :], bias_sb[:, :])
            # dummy activation takes the one-time ACT table-load pseudo-inst
            act_warm = consts.tile([128, 1], f32)
            nc.scalar.activation(act_warm[:, :], bias_act[:, 0:1], func=Act.Tanh)
            # give ACT an early observation of the PE prelude
            act_warm2 = consts.tile([128, 1], f32)
            nc.scalar.copy(act_warm2[:, :], psd[:, 0:1])

            for pr in range(N_PR):
                for b in range(BPC):
                    enc_tile = encp.tile([128, KC, SG2], bf16)
                    nc.sync.dma_start(out=enc_tile[:], in_=enc_d[b, pr, :, :, :])
                    # spare last column keeps the claim write disjoint from tanh
                    en_tile = enp.tile([128, DC, SG2 + 1], bf16)
                    # claim the recycled slot: carries the slot-release wait alone
                    nc.scalar.copy(en_tile[:, 0, SG2 : SG2 + 1], bias_act[:, 0:1])
                    scores_a = psp.tile([128, SG], f32, tag="sc", name="sca", bufs=3)
                    scores_b = psp.tile([128, SG], f32, tag="sc", name="scb", bufs=3)
                    for c in range(DC):
                        pp = psp.tile([128, 2, SG], f32, tag="proj", name="pp", bufs=2)
                        for h in range(2):
                            for k in range(KC):
                                nc.tensor.matmul(
                                    pp[:, h, :],
                                    pk[:, k, c * 128 : (c + 1) * 128],
                                    enc_tile[:, k, h * SG : (h + 1) * SG],
                                    start=(k == 0),
                                    stop=(k == KC - 1),
                                )
                        nc.scalar.activation(
                            out=en_tile[:, c, 0:SG2],
                            in_=pp[:, :, :],
                            func=Act.Tanh,
                            bias=bias_act[:, c * BPC + b : c * BPC + b + 1],
                        )
                        nc.tensor.matmul(
                            scores_a[32 * b : 32 * b + 1, :],
                            pk[:, c, _V0 : _V0 + 1],
                            en_tile[:, c, 0:SG],
                            start=(c == 0),
                            stop=(c == DC - 1),
                            tile_position=(0, 32 * b),
                        )
                        nc.tensor.matmul(
                            scores_b[32 * b : 32 * b + 1, :],
                            pk[:, c, _V0 : _V0 + 1],
                            en_tile[:, c, SG:SG2],
                            start=(c == 0),
                            stop=(c == DC - 1),
                            tile_position=(0, 32 * b),
                        )
                    sg = 2 * pr
                    nc.scalar.activation(
                        out=expd[32 * b : 32 * b + 1, sg * SG : (sg + 1) * SG],
                        in_=scores_a[32 * b : 32 * b + 1, :],
                        func=Act.Exp,
                        accum_out=sums8[32 * b : 32 * b + 1, sg : sg + 1],
                    )
                    nc.scalar.activation(
                        out=expd[32 * b : 32 * b + 1, (sg + 1) * SG : (sg + 2) * SG],
                        in_=scores_b[32 * b : 32 * b + 1, :],
                        func=Act.Exp,
                        accum_out=sums8[32 * b : 32 * b + 1, sg + 1 : sg + 2],
                    )

            sums = smp.tile([128, 1], f32)
            nc.vector.reduce_sum(sums[:, :], sums8[:, :], axis=mybir.AxisListType.X)
            recip = smp.tile([128, 1], f32)
            nc.vector.reciprocal(recip[:, :], sums[:, :])
            out_sb = smp.tile([128, S], f32)
            nc.vector.tensor_scalar_mul(
                out=out_sb[:, :], in0=expd[:, :], scalar1=recip[:, :]
            )
            # SWDGE: its DMA lanes are otherwise unused, so no lane-reuse wait
            nc.gpsimd.dma_start(out=out_d[:, :], in_=out_sb[0:128:32, :])

    _split_multi_waits(nc)
    return nc


def _split_multi_waits(nc):
    """This walrus build allows ONE sync wait per instruction. The kernel body
    is engineered to respect that; Tile's auto-emitted tail drain is not (it
    waits on every processor). Split any multi-wait instruction into a chain
    of single-wait drains on the same engine followed by the original."""
    from concourse import mybir

    for bb in nc.main_func.blocks:
        new_insts = []
        for ins in bb.instructions:
            si = getattr(ins, "sync_info", None)
            if si is not None and si.on_wait and len(si.on_wait) > 1:
                waits = list(si.on_wait)
                for w in waits[:-1]:
                    d = mybir.InstDrain(
                        name=nc.get_next_instruction_name(),
                        ins=[],
                        outs=[],
                        bass_is_fusable=False,
                    )
                    d.engine = ins.engine
                    d.sync_info = mybir.SyncInfo(on_wait=[w], on_update=[])
                    nc.register_instruction(d)
                    new_insts.append(d)
                si.on_wait = waits[-1:]
            new_insts.append(ins)
        bb.instructions[:] = new_insts


def _get_nc():
    global _nc_cache
    if _nc_cache is None:
        _nc_cache = _build_bass()
    return _nc_cache


def _prep_in_maps(decoder_hidden, encoder_outputs, W_attn, b_attn, v):
    decoder_hidden = np.asarray(decoder_hidden, dtype=np.float32)
    encoder_outputs = np.asarray(encoder_outputs, dtype=np.float32)
    W_attn = np.asarray(W_attn, dtype=np.float32)
    b_attn = np.asarray(b_attn, dtype=np.float32)
    v = np.asarray(v, dtype=np.float32)

    W_h = W_attn[:, :DEC]           # [d_out, d_in]
    W_e = W_attn[:, DEC:]           # [d_out, e]

    pk_base = np.zeros((128, KC, NPK), dtype=_BF16)
    pk_base[:, :, _WE0 : _WE0 + DEC] = (
        W_e.T.astype(_BF16).reshape(KC, 128, DEC).transpose(1, 0, 2)
    )
    pk_base[:, :, _WH0 : _WH0 + DEC] = (
        W_h.T.astype(_BF16).reshape(KC, 128, DEC).transpose(1, 0, 2)
    )
    pk_base[:, :, _V0] = v.astype(_BF16).reshape(DC, 128).T
    pk_base[:, :, _B0] = b_attn.astype(_BF16).reshape(DC, 128).T

    # [B, S, E] -> [B, N_PR, 128(p=e%128), KC(e//128), SG2(s)] in bf16
    enc_bt = np.ascontiguousarray(
        encoder_outputs.reshape(B, N_PR, SG2, KC, 128)
        .transpose(0, 1, 4, 3, 2)
        .astype(_BF16)
    )

    in_maps = []
    for core in range(N_CORES):
        sl = slice(core * BPC, (core + 1) * BPC)
        h = decoder_hidden[sl]                                   # [BPC, 512]
        pk = pk_base.copy()
        pk[:, :, _H0 : _H0 + BPC] = (
            h.astype(_BF16).reshape(BPC, KC, 128).transpose(2, 1, 0)
        )
        in_maps.append({"enc_t": enc_bt[sl], "pk": pk})
    return in_maps


def _ensure_ntff_hook():
    """The agent image's ``antenv`` lacks ``axon_hooks``; synthesize it with a
    ctypes-based NTFF profile hook against the injected libaxon (trace runs only)."""
    try:
        from antenv.axon_hooks import get_axon_ntff_profile_hook  # noqa: F401

        return
    except ImportError:
        pass

    import contextlib
    import ctypes
    import types

    so_path = "/opt/axon/libaxon_pjrt.so"
    hook = None
    if os.path.exists(so_path):
        lib = ctypes.CDLL(so_path)
        if hasattr(lib, "axon_start_nrt_profile"):
            lib.axon_start_nrt_profile.argtypes = [
                ctypes.POINTER(ctypes.c_int64),
                ctypes.c_size_t,
            ]
            lib.axon_start_nrt_profile.restype = ctypes.c_int64
            lib.axon_stop_nrt_profile.argtypes = [ctypes.c_char_p]
            lib.axon_stop_nrt_profile.restype = ctypes.c_int64

            @contextlib.contextmanager
            def _hook(output_dir, device_ids):
                import jax

                jax.devices()
                if device_ids:
                    ids = (ctypes.c_int64 * len(device_ids))(*device_ids)
                    rc = lib.axon_start_nrt_profile(ids, len(device_ids))
                else:
                    rc = lib.axon_start_nrt_profile(None, 0)
                if rc != 0:
                    raise RuntimeError(f"axon_start_nrt_profile rc={rc}")
                try:
                    yield
                finally:
                    n = lib.axon_stop_nrt_profile(str(output_dir).encode())
                    if n <= 0:
                        print(f"ntff capture wrote {n} files", file=sys.stderr)

            hook = _hook

    holder = {"h": hook}
    mod = types.ModuleType("antenv.axon_hooks")
    mod.get_axon_ntff_profile_hook = lambda: holder["h"]
    mod.set_axon_ntff_profile_hook = lambda h: holder.__setitem__("h", h)
    sys.modules["antenv.axon_hooks"] = mod
    import antenv

    antenv.axon_hooks = mod


def kernel(decoder_hidden, encoder_outputs, W_attn, b_attn, v):
    global last_results
    import concourse.bass_utils as bass_utils
    from concourse.bass_utils import run_bass_kernel_spmd

    nc = _get_nc()
    in_maps = _prep_in_maps(decoder_hidden, encoder_outputs, W_attn, b_attn, v)

    trace = os.environ.get("BAHDANAU_TRACE", "0") == "1"
    kwargs = {}
    if trace:
        _ensure_ntff_hook()
        bass_utils.upload_artifacts = lambda tmpdir: str(tmpdir)  # no bucket here
        kwargs["trace"] = True
        tmpdir = os.environ.get("BAHDANAU_TRACE_DIR")
        if tmpdir:
            import uuid

            tmpdir = os.path.join(tmpdir, uuid.uuid4().hex[:8])
            os.makedirs(tmpdir, exist_ok=True)
            kwargs["tmpdir"] = tmpdir

    res = run_bass_kernel_spmd(nc, in_maps, core_ids=list(range(N_CORES)), **kwargs)
    last_results = res
    out = np.concatenate([res.results[c]["out"] for c in range(N_CORES)], axis=0)
    return out.astype(np.float32)


# revision 25
# speedup vs baseline: 1.1347x; 1.0051x over previous
"""Bahdanau attention kernel for Trainium2 (8 NeuronCores, data-parallel over batch).

Computes, for each batch row b:
    energy  = tanh(enc[b] @ W_e.T + (h[b] @ W_h.T) + b_attn)   # [S, DEC]
    scores  = energy @ v                                        # [S]
    out[b]  = softmax(scores)

Shapes (hardcoded): B=32, S=4096, ENC=512, DEC=512. 8 cores, 4 batch rows/core.

Device-side design (per core):
  - encoder outputs are fed host-pre-tiled as [b, sg_pair, p, k, s] (bf16) so
    the contraction dim e lands on SBUF partitions with no on-chip transposes
    and each DMA reads 8KB contiguous per partition.
  - main matmul: proj[d_chunk(128), s(512)] += W_eT[e_chunk, d_chunk].T @ enc[e_chunk, s]
  - ACT fuses bias add (per-partition) + tanh over a 2-bank [128,1024] PSUM pair
  - v-dot: scores[1, s] += v[d_chunk].T @ energy[d_chunk, s]; batch b parked at
    PSUM partition 32*b of its own scores bank.
  - ACT exp with accum_out yields per-partition exp sums; DVE normalizes.
  - All constants ride in ONE packed bf16 param; this walrus build allows one
    sync wait per instruction, so the dataflow is engineered for single-wait
    instructions and a post-pass splits any leftovers into wait-only drains.
"""

import os
import sys

import numpy as np

try:
    import concourse.bass as bass  # noqa: F401
except ImportError:  # toolchain lives in the trn_rl repo
    for p in ("/opt/trn_rl_repo", "/root/.axon_site/_ro/trn_rl_repo"):
        if os.path.isdir(p) and p not in sys.path:
            sys.path.insert(0, p)
    import concourse.bass as bass  # noqa: F401

import ml_dtypes

B, S, ENC, DEC = 32, 4096, 512, 512
N_CORES = 8
BPC = B // N_CORES          # batch rows per core
SG = 512                    # s-columns per matmul group
SG2 = 2 * SG                # s-columns per DMA tile / tanh
N_PR = S // SG2             # 4 s-group pairs
KC = ENC // 128             # 4 contraction chunks
DC = DEC // 128             # 4 output-dim chunks

# packed constant layout (bf16): [128, KC, NPK]
_WE0 = 0            # W_e.T            cols [0, 512)
_WH0 = DEC          # W_h.T            cols [512, 1024)
_V0 = 2 * DEC       # v                col  1024
_H0 = 2 * DEC + 1   # decoder hidden   cols [1025, 1025+BPC)
_B0 = _H0 + BPC     # b_attn           col  1029
NPK = _B0 + 1

_BF16 = ml_dtypes.bfloat16

_nc_cache = None
last_results = None         # BassKernelResults of the most recent run (for test.py)


def _build_bass():
    import concourse.tile as tile
    from concourse import mybir

    f32 = mybir.dt.float32
    bf16 = mybir.dt.bfloat16
    Act = mybir.ActivationFunctionType

    nc = bass.Bass()

    enc_d = nc.declare_dram_parameter(
        "enc_t", [BPC, N_PR, 128, KC, SG2], bf16, isOutput=False
    )
    pk_d = nc.declare_dram_parameter("pk", [128, KC, NPK], bf16, isOutput=False)
    out_d = nc.declare_dram_parameter("out", [BPC, S], f32, isOutput=True)

    with tile.TileContext(nc) as tc:
        with (
            tc.tile_pool(name="consts", bufs=1) as consts,
            tc.tile_pool(name="encp", bufs=BPC * N_PR) as encp,
            tc.tile_pool(name="enp", bufs=3) as enp,
            tc.tile_pool(name="psp", bufs=2, space="PSUM") as psp,
            tc.tile_pool(name="smp", bufs=1) as smp,
        ):
            pk = consts.tile([128, KC, NPK], bf16)
            nc.sync.dma_start(out=pk[:], in_=pk_d[:, :, :])

            expd = smp.tile([128, S], f32)
            nc.vector.memset(expd[:, :], 0.0)
            sums8 = smp.tile([128, 2 * N_PR], f32)
            nc.vector.memset(sums8[:, :], 1.0)

            # decoder projection: psd[:, c*BPC+b] = (W_h @ h_b)[c*128:(c+1)*128]
            # batched over the 4 batch rows (N=4 matmuls)
            psd = psp.tile([128, DC * BPC], f32, tag="sc", name="psd", bufs=4)
            for c in range(DC):
                for k in range(KC):
                    nc.tensor.matmul(
                        psd[:, c * BPC : (c + 1) * BPC],
                        pk[:, k, _WH0 + c * 128 : _WH0 + (c + 1) * 128],
                        pk[:, k, _H0 : _H0 + BPC],
                        start=(k == 0),
                        stop=(k == KC - 1),
                    )

            # f32 copy of b_attn columns; also absorbs the pk DMA into DVE's clock
            b_cols = consts.tile([128, DC], f32)
            nc.vector.tensor_copy(b_cols[:, :], pk[:, :, _B0])
            bias_sb = consts.tile([128, DC * BPC], f32)
            for c in range(DC):
                nc.vector.tensor_scalar_add(
                    out=bias_sb[:, c * BPC : (c + 1) * BPC],
                    in0=psd[:, c * BPC : (c + 1) * BPC],
                    scalar1=b_cols[:, c : c + 1],
                )
            # re-materialize the bias through ACT: tanh's bias dep becomes a
            # same-queue edge (single-sync-wait constraint)
            bias_act = consts.tile([128, DC * BPC], f32)
            nc.scalar.copy(bias_act[:, :], bias_sb[:, :])
            # dummy activation takes the one-time ACT table-load pseudo-inst
            act_warm = consts.tile([128, 1], f32)
            nc.scalar.activation(act_warm[:, :], bias_act[:, 0:1], func=Act.Tanh)
            # give ACT an early observation of the PE prelude
            act_warm2 = consts.tile([128, 1], f32)
            nc.scalar.copy(act_warm2[:, :], psd[:, 0:1])

            for pr in range(N_PR):
                for b in range(BPC):
                    enc_tile = encp.tile([128, KC, SG2], bf16)
                    nc.sync.dma_start(
                        out=enc_tile[:, 0:2, :], in_=enc_d[b, pr, :, 0:2, :]
                    )
                    nc.sync.dma_start(
                        out=enc_tile[:, 2:4, :], in_=enc_d[b, pr, :, 2:4, :]
                    )
                    # spare last column keeps the claim write disjoint from tanh
                    en_tile = enp.tile([128, DC, SG2 + 1], bf16)
                    # claim the recycled slot: carries the slot-release wait alone
                    nc.scalar.copy(en_tile[:, 0, SG2 : SG2 + 1], bias_act[:, 0:1])
                    scores_a = psp.tile([128, SG], f32, tag="sc", name="sca", bufs=4)
                    scores_b = psp.tile([128, SG], f32, tag="sc", name="scb", bufs=4)
                    for c in range(DC):
                        pp = psp.tile([128, 2, SG], f32, tag="proj", name="pp", bufs=2)
                        for h in range(2):
                            for k in range(KC):
                                nc.tensor.matmul(
                                    pp[:, h, :],
                                    pk[:, k, c * 128 : (c + 1) * 128],
                                    enc_tile[:, k, h * SG : (h + 1) * SG],
                                    start=(k == 0),
                                    stop=(k == KC - 1),
                                )
                        nc.scalar.activation(
                            out=en_tile[:, c, 0:SG2],
                            in_=pp[:, :, :],
                            func=Act.Tanh,
                            bias=bias_act[:, c * BPC + b : c * BPC + b + 1],
                        )
                        nc.tensor.matmul(
                            scores_a[32 * b : 32 * b + 1, :],
                            pk[:, c, _V0 : _V0 + 1],
                            en_tile[:, c, 0:SG],
                            start=(c == 0),
                            stop=(c == DC - 1),
                            tile_position=(0, 32 * b),
                        )
                        nc.tensor.matmul(
                            scores_b[32 * b : 32 * b + 1, :],
                            pk[:, c, _V0 : _V0 + 1],
                            en_tile[:, c, SG:SG2],
                            start=(c == 0),
                            stop=(c == DC - 1),
                            tile_position=(0, 32 * b),
                        )
                    sg = 2 * pr
                    nc.scalar.activation(
                        out=expd[32 * b : 32 * b + 1, sg * SG : (sg + 1) * SG],
                        in_=scores_a[32 * b : 32 * b + 1, :],
                        func=Act.Exp,
                        accum_out=sums8[32 * b : 32 * b + 1, sg : sg + 1],
                    )
                    nc.scalar.activation(
                        out=expd[32 * b : 32 * b + 1, (sg + 1) * SG : (sg + 2) * SG],
                        in_=scores_b[32 * b : 32 * b + 1, :],
                        func=Act.Exp,
                        accum_out=sums8[32 * b : 32 * b + 1, sg + 1 : sg + 2],
                    )

            sums = smp.tile([128, 1], f32)
            nc.vector.reduce_sum(sums[:, :], sums8[:, :], axis=mybir.AxisListType.X)
            recip = smp.tile([128, 1], f32)
            nc.vector.reciprocal(recip[:, :], sums[:, :])
            out_sb = smp.tile([128, S], f32)
            nc.vector.tensor_scalar_mul(
                out=out_sb[:, :], in0=expd[:, :], scalar1=recip[:, :]
            )
            # SWDGE: its DMA lanes are otherwise unused, so no lane-reuse wait
            nc.gpsimd.dma_start(out=out_d[:, :], in_=out_sb[0:128:32, :])

    _split_multi_waits(nc)
    return nc


def _split_multi_waits(nc):
    """This walrus build allows ONE sync wait per instruction. The kernel body
    is engineered to respect that; Tile's auto-emitted tail drain is not (it
    waits on every processor). Split any multi-wait instruction into a chain
    of single-wait drains on the same engine followed by the original."""
    from concourse import mybir

    for bb in nc.main_func.blocks:
        new_insts = []
        for ins in bb.instructions:
            si = getattr(ins, "sync_info", None)
            if si is not None and si.on_wait and len(si.on_wait) > 1:
                waits = list(si.on_wait)
                for w in waits[:-1]:
                    d = mybir.InstDrain(
                        name=nc.get_next_instruction_name(),
                        ins=[],
                        outs=[],
                        bass_is_fusable=False,
                    )
                    d.engine = ins.engine
                    d.sync_info = mybir.SyncInfo(on_wait=[w], on_update=[])
                    nc.register_instruction(d)
                    new_insts.append(d)
                si.on_wait = waits[-1:]
            new_insts.append(ins)
        bb.instructions[:] = new_insts


def _get_nc():
    global _nc_cache
    if _nc_cache is None:
        _nc_cache = _build_bass()
    return _nc_cache


def _prep_in_maps(decoder_hidden, encoder_outputs, W_attn, b_attn, v):
    decoder_hidden = np.asarray(decoder_hidden, dtype=np.float32)
    encoder_outputs = np.asarray(encoder_outputs, dtype=np.float32)
    W_attn = np.asarray(W_attn, dtype=np.float32)
    b_attn = np.asarray(b_attn, dtype=np.float32)
    v = np.asarray(v, dtype=np.float32)

    W_h = W_attn[:, :DEC]           # [d_out, d_in]
    W_e = W_attn[:, DEC:]           # [d_out, e]

    pk_base = np.zeros((128, KC, NPK), dtype=_BF16)
    pk_base[:, :, _WE0 : _WE0 + DEC] = (
        W_e.T.astype(_BF16).reshape(KC, 128, DEC).transpose(1, 0, 2)
    )
    pk_base[:, :, _WH0 : _WH0 + DEC] = (
        W_h.T.astype(_BF16).reshape(KC, 128, DEC).transpose(1, 0, 2)
    )
    pk_base[:, :, _V0] = v.astype(_BF16).reshape(DC, 128).T
    pk_base[:, :, _B0] = b_attn.astype(_BF16).reshape(DC, 128).T

    # [B, S, E] -> [B, N_PR, 128(p=e%128), KC(e//128), SG2(s)] in bf16
    enc_bt = np.ascontiguousarray(
        encoder_outputs.reshape(B, N_PR, SG2, KC, 128)
        .transpose(0, 1, 4, 3, 2)
        .astype(_BF16)
    )

    in_maps = []
    for core in range(N_CORES):
        sl = slice(core * BPC, (core + 1) * BPC)
        h = decoder_hidden[sl]                                   # [BPC, 512]
        pk = pk_base.copy()
        pk[:, :, _H0 : _H0 + BPC] = (
            h.astype(_BF16).reshape(BPC, KC, 128).transpose(2, 1, 0)
        )
        in_maps.append({"enc_t": enc_bt[sl], "pk": pk})
    return in_maps


def _ensure_ntff_hook():
    """The agent image's ``antenv`` lacks ``axon_hooks``; synthesize it with a
    ctypes-based NTFF profile hook against the injected libaxon (trace runs only)."""
    try:
        from antenv.axon_hooks import get_axon_ntff_profile_hook  # noqa: F401

        return
    except ImportError:
        pass

    import contextlib
    import ctypes
    import types

    so_path = "/opt/axon/libaxon_pjrt.so"
    hook = None
    if os.path.exists(so_path):
        lib = ctypes.CDLL(so_path)
        if hasattr(lib, "axon_start_nrt_profile"):
            lib.axon_start_nrt_profile.argtypes = [
                ctypes.POINTER(ctypes.c_int64),
                ctypes.c_size_t,
            ]
            lib.axon_start_nrt_profile.restype = ctypes.c_int64
            lib.axon_stop_nrt_profile.argtypes = [ctypes.c_char_p]
            lib.axon_stop_nrt_profile.restype = ctypes.c_int64

            @contextlib.contextmanager
            def _hook(output_dir, device_ids):
                import jax

                jax.devices()
                if device_ids:
                    ids = (ctypes.c_int64 * len(device_ids))(*device_ids)
                    rc = lib.axon_start_nrt_profile(ids, len(device_ids))
                else:
                    rc = lib.axon_start_nrt_profile(None, 0)
                if rc != 0:
                    raise RuntimeError(f"axon_start_nrt_profile rc={rc}")
                try:
                    yield
                finally:
                    n = lib.axon_stop_nrt_profile(str(output_dir).encode())
                    if n <= 0:
                        print(f"ntff capture wrote {n} files", file=sys.stderr)

            hook = _hook

    holder = {"h": hook}
    mod = types.ModuleType("antenv.axon_hooks")
    mod.get_axon_ntff_profile_hook = lambda: holder["h"]
    mod.set_axon_ntff_profile_hook = lambda h: holder.__setitem__("h", h)
    sys.modules["antenv.axon_hooks"] = mod
    import antenv

    antenv.axon_hooks = mod


def kernel(decoder_hidden, encoder_outputs, W_attn, b_attn, v):
    global last_results
    import concourse.bass_utils as bass_utils
    from concourse.bass_utils import run_bass_kernel_spmd

    nc = _get_nc()
    in_maps = _prep_in_maps(decoder_hidden, encoder_outputs, W_attn, b_attn, v)

    trace = os.environ.get("BAHDANAU_TRACE", "0") == "1"
    kwargs = {}
    if trace:
        _ensure_ntff_hook()
        bass_utils.upload_artifacts = lambda tmpdir: str(tmpdir)  # no bucket here
        kwargs["trace"] = True
        tmpdir = os.environ.get("BAHDANAU_TRACE_DIR")
        if tmpdir:
            import uuid

            tmpdir = os.path.join(tmpdir, uuid.uuid4().hex[:8])
            os.makedirs(tmpdir, exist_ok=True)
            kwargs["tmpdir"] = tmpdir

    res = run_bass_kernel_spmd(nc, in_maps, core_ids=list(range(N_CORES)), **kwargs)
    last_results = res
    out = np.concatenate([res.results[c]["out"] for c in range(N_CORES)], axis=0)
    return out.astype(np.float32)


# revision 26
# speedup vs baseline: 1.1381x; 1.0030x over previous
"""Bahdanau attention kernel for Trainium2 (8 NeuronCores, data-parallel over batch).

Computes, for each batch row b:
    energy  = tanh(enc[b] @ W_e.T + (h[b] @ W_h.T) + b_attn)   # [S, DEC]
    scores  = energy @ v                                        # [S]
    out[b]  = softmax(scores)

Shapes (hardcoded): B=32, S=4096, ENC=512, DEC=512. 8 cores, 4 batch rows/core.

Device-side design (per core):
  - encoder outputs are fed host-pre-tiled as [b, sg_pair, p, k, s] (bf16) so
    the contraction dim e lands on SBUF partitions with no on-chip transposes
    and each DMA reads 8KB contiguous per partition.
  - main matmul: proj[d_chunk(128), s(512)] += W_eT[e_chunk, d_chunk].T @ enc[e_chunk, s]
  - ACT fuses bias add (per-partition) + tanh over a 2-bank [128,1024] PSUM pair
  - v-dot: scores[1, s] += v[d_chunk].T @ energy[d_chunk, s]; batch b parked at
    PSUM partition 32*b of its own scores bank.
  - ACT exp with accum_out yields per-partition exp sums; DVE normalizes.
  - All constants ride in ONE packed bf16 param; this walrus build allows one
    sync wait per instruction, so the dataflow is engineered for single-wait
    instructions and a post-pass splits any leftovers into wait-only drains.
"""

import os
import sys

import numpy as np

try:
    import concourse.bass as bass  # noqa: F401
except ImportError:  # toolchain lives in the trn_rl repo
    for p in ("/opt/trn_rl_repo", "/root/.axon_site/_ro/trn_rl_repo"):
        if os.path.isdir(p) and p not in sys.path:
            sys.path.insert(0, p)
    import concourse.bass as bass  # noqa: F401

import ml_dtypes

B, S, ENC, DEC = 32, 4096, 512, 512
N_CORES = 8
BPC = B // N_CORES          # batch rows per core
SG = 512                    # s-columns per matmul group
SG2 = 2 * SG                # s-columns per DMA tile / tanh
N_PR = S // SG2             # 4 s-group pairs
KC = ENC // 128             # 4 contraction chunks
DC = DEC // 128             # 4 output-dim chunks

# packed constant layout (bf16): [128, KC, NPK]
_WE0 = 0            # W_e.T            cols [0, 512)
_WH0 = DEC          # W_h.T            cols [512, 1024)
_V0 = 2 * DEC       # v                col  1024
_H0 = 2 * DEC + 1   # decoder hidden   cols [1025, 1025+BPC)
_B0 = _H0 + BPC     # b_attn           col  1029
NPK = _B0 + 1

_BF16 = ml_dtypes.bfloat16

_nc_cache = None
last_results = None         # BassKernelResults of the most recent run (for test.py)


def _build_bass():
    import concourse.tile as tile
    from concourse import mybir

    f32 = mybir.dt.float32
    bf16 = mybir.dt.bfloat16
    Act = mybir.ActivationFunctionType

    nc = bass.Bass()

    enc_d = nc.declare_dram_parameter(
        "enc_t", [BPC, N_PR, 128, KC, SG2], bf16, isOutput=False
    )
    pk_d = nc.declare_dram_parameter("pk", [128, KC, NPK], bf16, isOutput=False)
    out_d = nc.declare_dram_parameter("out", [BPC, S], f32, isOutput=True)

    with tile.TileContext(nc) as tc:
        with (
            tc.tile_pool(name="consts", bufs=1) as consts,
            tc.tile_pool(name="encp", bufs=4) as encp,
            tc.tile_pool(name="enp", bufs=3) as enp,
            tc.tile_pool(name="psp", bufs=2, space="PSUM") as psp,
            tc.tile_pool(name="smp", bufs=1) as smp,
        ):
            pk = consts.tile([128, KC, NPK], bf16)
            nc.sync.dma_start(out=pk[:], in_=pk_d[:, :, :])

            expd = smp.tile([128, S], f32)
            nc.vector.memset(expd[:, :], 0.0)
            sums8 = smp.tile([128, 2 * N_PR], f32)
            nc.vector.memset(sums8[:, :], 1.0)

            # decoder projection: psd[:, c*BPC+b] = (W_h @ h_b)[c*128:(c+1)*128]
            # batched over the 4 batch rows (N=4 matmuls)
            psd = psp.tile([128, DC * BPC], f32, tag="sc", name="psd", bufs=4)
            for c in range(DC):
                for k in range(KC):
                    nc.tensor.matmul(
                        psd[:, c * BPC : (c + 1) * BPC],
                        pk[:, k, _WH0 + c * 128 : _WH0 + (c + 1) * 128],
                        pk[:, k, _H0 : _H0 + BPC],
                        start=(k == 0),
                        stop=(k == KC - 1),
                    )

            # f32 copy of b_attn columns; also absorbs the pk DMA into DVE's clock
            b_cols = consts.tile([128, DC], f32)
            nc.vector.tensor_copy(b_cols[:, :], pk[:, :, _B0])
            bias_sb = consts.tile([128, DC * BPC], f32)
            for c in range(DC):
                nc.vector.tensor_scalar_add(
                    out=bias_sb[:, c * BPC : (c + 1) * BPC],
                    in0=psd[:, c * BPC : (c + 1) * BPC],
                    scalar1=b_cols[:, c : c + 1],
                )
            # re-materialize the bias through ACT: tanh's bias dep becomes a
            # same-queue edge (single-sync-wait constraint)
            bias_act = consts.tile([128, DC * BPC], f32)
            nc.scalar.copy(bias_act[:, :], bias_sb[:, :])
            # dummy activation takes the one-time ACT table-load pseudo-inst
            act_warm = consts.tile([128, 1], f32)
            nc.scalar.activation(act_warm[:, :], bias_act[:, 0:1], func=Act.Tanh)
            # give ACT an early observation of the PE prelude
            act_warm2 = consts.tile([128, 1], f32)
            nc.scalar.copy(act_warm2[:, :], psd[:, 0:1])

            for pr in range(N_PR):
                for b in range(BPC):
                    enc_tile = encp.tile([128, KC, SG2], bf16)
                    nc.sync.dma_start(
                        out=enc_tile[:, 0:2, :], in_=enc_d[b, pr, :, 0:2, :]
                    )
                    nc.sync.dma_start(
                        out=enc_tile[:, 2:4, :], in_=enc_d[b, pr, :, 2:4, :]
                    )
                    # spare last column keeps the claim write disjoint from tanh
                    en_tile = enp.tile([128, DC, SG2 + 1], bf16)
                    # claim the recycled slot: carries the slot-release wait alone
                    nc.scalar.copy(en_tile[:, 0, SG2 : SG2 + 1], bias_act[:, 0:1])
                    scores_a = psp.tile([128, SG], f32, tag="sc", name="sca", bufs=4)
                    scores_b = psp.tile([128, SG], f32, tag="sc", name="scb", bufs=4)
                    for c in range(DC):
                        pp = psp.tile([128, 2, SG], f32, tag="proj", name="pp", bufs=2)
                        for h in range(2):
                            for k in range(KC):
                                nc.tensor.matmul(
                                    pp[:, h, :],
                                    pk[:, k, c * 128 : (c + 1) * 128],
                                    enc_tile[:, k, h * SG : (h + 1) * SG],
                                    start=(k == 0),
                                    stop=(k == KC - 1),
                                )
                        nc.scalar.activation(
                            out=en_tile[:, c, 0:SG2],
                            in_=pp[:, :, :],
                            func=Act.Tanh,
                            bias=bias_act[:, c * BPC + b : c * BPC + b + 1],
                        )
                        nc.tensor.matmul(
                            scores_a[32 * b : 32 * b + 1, :],
                            pk[:, c, _V0 : _V0 + 1],
                            en_tile[:, c, 0:SG],
                            start=(c == 0),
                            stop=(c == DC - 1),
                            tile_position=(0, 32 * b),
                        )
                        nc.tensor.matmul(
                            scores_b[32 * b : 32 * b + 1, :],
                            pk[:, c, _V0 : _V0 + 1],
                            en_tile[:, c, SG:SG2],
                            start=(c == 0),
                            stop=(c == DC - 1),
                            tile_position=(0, 32 * b),
                        )
                    sg = 2 * pr
                    nc.scalar.activation(
                        out=expd[32 * b : 32 * b + 1, sg * SG : (sg + 1) * SG],
                        in_=scores_a[32 * b : 32 * b + 1, :],
                        func=Act.Exp,
                        accum_out=sums8[32 * b : 32 * b + 1, sg : sg + 1],
                    )
                    nc.scalar.activation(
                        out=expd[32 * b : 32 * b + 1, (sg + 1) * SG : (sg + 2) * SG],
                        in_=scores_b[32 * b : 32 * b + 1, :],
                        func=Act.Exp,
                        accum_out=sums8[32 * b : 32 * b + 1, sg + 1 : sg + 2],
                    )

            sums = smp.tile([128, 1], f32)
            recip = smp.tile([128, 1], f32)
            out_sb = smp.tile([128, S], f32)
            for b in range(BPC):
                r = slice(32 * b, 32 * b + 1)
                nc.vector.reduce_sum(
                    sums[r, :], sums8[r, :], axis=mybir.AxisListType.X
                )
                nc.vector.reciprocal(recip[r, :], sums[r, :])
                nc.vector.tensor_scalar_mul(
                    out=out_sb[r, :], in0=expd[r, :], scalar1=recip[r, :]
                )
                nc.sync.dma_start(out=out_d[b : b + 1, :], in_=out_sb[r, :])

    _split_multi_waits(nc)
    return nc


def _split_multi_waits(nc):
    """This walrus build allows ONE sync wait per instruction. The kernel body
    is engineered to respect that; Tile's auto-emitted tail drain is not (it
    waits on every processor). Split any multi-wait instruction into a chain
    of single-wait drains on the same engine followed by the original."""
    from concourse import mybir

    for bb in nc.main_func.blocks:
        new_insts = []
        for ins in bb.instructions:
            si = getattr(ins, "sync_info", None)
            if si is not None and si.on_wait and len(si.on_wait) > 1:
                waits = list(si.on_wait)
                for w in waits[:-1]:
                    d = mybir.InstDrain(
                        name=nc.get_next_instruction_name(),
                        ins=[],
                        outs=[],
                        bass_is_fusable=False,
                    )
                    d.engine = ins.engine
                    d.sync_info = mybir.SyncInfo(on_wait=[w], on_update=[])
                    nc.register_instruction(d)
                    new_insts.append(d)
                si.on_wait = waits[-1:]
            new_insts.append(ins)
        bb.instructions[:] = new_insts


def _get_nc():
    global _nc_cache
    if _nc_cache is None:
        _nc_cache = _build_bass()
    return _nc_cache


def _prep_in_maps(decoder_hidden, encoder_outputs, W_attn, b_attn, v):
    decoder_hidden = np.asarray(decoder_hidden, dtype=np.float32)
    encoder_outputs = np.asarray(encoder_outputs, dtype=np.float32)
    W_attn = np.asarray(W_attn, dtype=np.float32)
    b_attn = np.asarray(b_attn, dtype=np.float32)
    v = np.asarray(v, dtype=np.float32)

    W_h = W_attn[:, :DEC]           # [d_out, d_in]
    W_e = W_attn[:, DEC:]           # [d_out, e]

    pk_base = np.zeros((128, KC, NPK), dtype=_BF16)
    pk_base[:, :, _WE0 : _WE0 + DEC] = (
        W_e.T.astype(_BF16).reshape(KC, 128, DEC).transpose(1, 0, 2)
    )
    pk_base[:, :, _WH0 : _WH0 + DEC] = (
        W_h.T.astype(_BF16).reshape(KC, 128, DEC).transpose(1, 0, 2)
    )
    pk_base[:, :, _V0] = v.astype(_BF16).reshape(DC, 128).T
    pk_base[:, :, _B0] = b_attn.astype(_BF16).reshape(DC, 128).T

    # [B, S, E] -> [B, N_PR, 128(p=e%128), KC(e//128), SG2(s)] in bf16
    enc_bt = np.ascontiguousarray(
        encoder_outputs.reshape(B, N_PR, SG2, KC, 128)
        .transpose(0, 1, 4, 3, 2)
        .astype(_BF16)
    )

    in_maps = []
    for core in range(N_CORES):
        sl = slice(core * BPC, (core + 1) * BPC)
        h = decoder_hidden[sl]                                   # [BPC, 512]
        pk = pk_base.copy()
        pk[:, :, _H0 : _H0 + BPC] = (
            h.astype(_BF16).reshape(BPC, KC, 128).transpose(2, 1, 0)
        )
        in_maps.append({"enc_t": enc_bt[sl], "pk": pk})
    return in_maps


def _ensure_ntff_hook():
    """The agent image's ``antenv`` lacks ``axon_hooks``; synthesize it with a
    ctypes-based NTFF profile hook against the injected libaxon (trace runs only)."""
    try:
        from antenv.axon_hooks import get_axon_ntff_profile_hook  # noqa: F401

        return
    except ImportError:
        pass

    import contextlib
    import ctypes
    import types

    so_path = "/opt/axon/libaxon_pjrt.so"
    hook = None
    if os.path.exists(so_path):
        lib = ctypes.CDLL(so_path)
        if hasattr(lib, "axon_start_nrt_profile"):
            lib.axon_start_nrt_profile.argtypes = [
                ctypes.POINTER(ctypes.c_int64),
                ctypes.c_size_t,
            ]
            lib.axon_start_nrt_profile.restype = ctypes.c_int64
            lib.axon_stop_nrt_profile.argtypes = [ctypes.c_char_p]
            lib.axon_stop_nrt_profile.restype = ctypes.c_int64

            @contextlib.contextmanager
            def _hook(output_dir, device_ids):
                import jax

                jax.devices()
                if device_ids:
                    ids = (ctypes.c_int64 * len(device_ids))(*device_ids)
                    rc = lib.axon_start_nrt_profile(ids, len(device_ids))
                else:
                    rc = lib.axon_start_nrt_profile(None, 0)
                if rc != 0:
                    raise RuntimeError(f"axon_start_nrt_profile rc={rc}")
                try:
                    yield
                finally:
                    n = lib.axon_stop_nrt_profile(str(output_dir).encode())
                    if n <= 0:
                        print(f"ntff capture wrote {n} files", file=sys.stderr)

            hook = _hook

    holder = {"h": hook}
    mod = types.ModuleType("antenv.axon_hooks")
    mod.get_axon_ntff_profile_hook = lambda: holder["h"]
    mod.set_axon_ntff_profile_hook = lambda h: holder.__setitem__("h", h)
    sys.modules["antenv.axon_hooks"] = mod
    import antenv

    antenv.axon_hooks = mod


def kernel(decoder_hidden, encoder_outputs, W_attn, b_attn, v):
    global last_results
    import concourse.bass_utils as bass_utils
    from concourse.bass_utils import run_bass_kernel_spmd

    nc = _get_nc()
    in_maps = _prep_in_maps(decoder_hidden, encoder_outputs, W_attn, b_attn, v)

    trace = os.environ.get("BAHDANAU_TRACE", "0") == "1"
    kwargs = {}
    if trace:
        _ensure_ntff_hook()
        bass_utils.upload_artifacts = lambda tmpdir: str(tmpdir)  # no bucket here
        kwargs["trace"] = True
        tmpdir = os.environ.get("BAHDANAU_TRACE_DIR")
        if tmpdir:
            import uuid

            tmpdir = os.path.join(tmpdir, uuid.uuid4().hex[:8])
            os.makedirs(tmpdir, exist_ok=True)
            kwargs["tmpdir"] = tmpdir

    res = run_bass_kernel_spmd(nc, in_maps, core_ids=list(range(N_CORES)), **kwargs)
    last_results = res
    out = np.concatenate([res.results[c]["out"] for c in range(N_CORES)], axis=0)
    return out.astype(np.float32)


# revision 28
# speedup vs baseline: 1.1596x; 1.0189x over previous
"""Bahdanau attention kernel for Trainium2 (8 NeuronCores, data-parallel over batch).

Computes, for each batch row b:
    energy  = tanh(enc[b] @ W_e.T + (h[b] @ W_h.T) + b_attn)   # [S, DEC]
    scores  = energy @ v                                        # [S]
    out[b]  = softmax(scores)

Shapes (hardcoded): B=32, S=4096, ENC=512, DEC=512. 8 cores, 4 batch rows/core.

Device-side design (per core):
  - encoder outputs are fed host-pre-tiled as [b, sg_pair, p, k, s] (bf16) so
    the contraction dim e lands on SBUF partitions with no on-chip transposes
    and each DMA reads 8KB contiguous per partition.
  - main matmul: proj[d_chunk(128), s(512)] += W_eT[e_chunk, d_chunk].T @ enc[e_chunk, s]
  - ACT fuses bias add (per-partition) + tanh over a 2-bank [128,1024] PSUM pair
  - v-dot: scores[1, s] += v[d_chunk].T @ energy[d_chunk, s]; batch b parked at
    PSUM partition 32*b of its own scores bank.
  - ACT exp with accum_out yields per-partition exp sums; DVE normalizes.
  - All constants ride in ONE packed bf16 param; this walrus build allows one
    sync wait per instruction, so the dataflow is engineered for single-wait
    instructions and a post-pass splits any leftovers into wait-only drains.
"""

import os
import sys

import numpy as np

try:
    import concourse.bass as bass  # noqa: F401
except ImportError:  # toolchain lives in the trn_rl repo
    for p in ("/opt/trn_rl_repo", "/root/.axon_site/_ro/trn_rl_repo"):
        if os.path.isdir(p) and p not in sys.path:
            sys.path.insert(0, p)
    import concourse.bass as bass  # noqa: F401

import ml_dtypes

B, S, ENC, DEC = 32, 4096, 512, 512
N_CORES = 8
BPC = B // N_CORES          # batch rows per core
SG = 512                    # s-columns per matmul group
SG2 = 2 * SG                # s-columns per DMA tile / tanh
N_PR = S // SG2             # 4 s-group pairs
KC = ENC // 128             # 4 contraction chunks
DC = DEC // 128             # 4 output-dim chunks

# packed constant layout (bf16): [128, KC, NPK]
_WE0 = 0            # W_e.T            cols [0, 512)
_WH0 = DEC          # W_h.T            cols [512, 1024)
_V0 = 2 * DEC       # v                col  1024
_H0 = 2 * DEC + 1   # decoder hidden   cols [1025, 1025+BPC)
_B0 = _H0 + BPC     # b_attn           col  1029
NPK = _B0 + 1

_BF16 = ml_dtypes.bfloat16

_nc_cache = None
last_results = None         # BassKernelResults of the most recent run (for test.py)


def _build_bass():
    import concourse.tile as tile
    from concourse import mybir

    f32 = mybir.dt.float32
    bf16 = mybir.dt.bfloat16
    Act = mybir.ActivationFunctionType

    nc = bass.Bass()

    enc_d = nc.declare_dram_parameter(
        "enc_t", [BPC, N_PR, 128, KC, SG2], bf16, isOutput=False
    )
    pk_d = nc.declare_dram_parameter("pk", [128, KC, NPK], bf16, isOutput=False)
    out_d = nc.declare_dram_parameter("out", [BPC, S], f32, isOutput=True)

    with tile.TileContext(nc) as tc:
        with (
            tc.tile_pool(name="consts", bufs=1) as consts,
            tc.tile_pool(name="encp", bufs=4) as encp,
            tc.tile_pool(name="enp", bufs=3) as enp,
            tc.tile_pool(name="psp", bufs=2, space="PSUM") as psp,
            tc.tile_pool(name="smp", bufs=1) as smp,
        ):
            pk = consts.tile([128, KC, NPK], bf16)
            nc.sync.dma_start(out=pk[:], in_=pk_d[:, :, :])

            expd = smp.tile([128, S], f32)
            nc.vector.memset(expd[:, :], 0.0)
            sums8 = smp.tile([128, 2 * N_PR], f32)
            nc.vector.memset(sums8[:, :], 1.0)

            # decoder projection: psd[:, c*BPC+b] = (W_h @ h_b)[c*128:(c+1)*128]
            # batched over the 4 batch rows (N=4 matmuls)
            psd = psp.tile([128, DC * BPC], f32, tag="sc", name="psd", bufs=4)
            for c in range(DC):
                for k in range(KC):
                    nc.tensor.matmul(
                        psd[:, c * BPC : (c + 1) * BPC],
                        pk[:, k, _WH0 + c * 128 : _WH0 + (c + 1) * 128],
                        pk[:, k, _H0 : _H0 + BPC],
                        start=(k == 0),
                        stop=(k == KC - 1),
                    )

            # f32 copy of b_attn columns; also absorbs the pk DMA into DVE's clock
            b_cols = consts.tile([128, DC], f32)
            nc.vector.tensor_copy(b_cols[:, :], pk[:, :, _B0])
            bias_sb = consts.tile([128, DC * BPC], f32)
            for c in range(DC):
                nc.vector.tensor_scalar_add(
                    out=bias_sb[:, c * BPC : (c + 1) * BPC],
                    in0=psd[:, c * BPC : (c + 1) * BPC],
                    scalar1=b_cols[:, c : c + 1],
                )
            # re-materialize the bias through ACT: tanh's bias dep becomes a
            # same-queue edge (single-sync-wait constraint)
            bias_act = consts.tile([128, DC * BPC], f32)
            nc.scalar.copy(bias_act[:, :], bias_sb[:, :])
            # dummy activation takes the one-time ACT table-load pseudo-inst
            act_warm = consts.tile([128, 1], f32)
            nc.scalar.activation(act_warm[:, :], bias_act[:, 0:1], func=Act.Tanh)
            # give ACT an early observation of the PE prelude
            act_warm2 = consts.tile([128, 1], f32)
            nc.scalar.copy(act_warm2[:, :], psd[:, 0:1])

            # Serialize the first four enc DMAs behind the pk load and each
            # other (an ACT claim-write into each tile's region makes the DMA
            # wait on it): otherwise 4+ MB of enc traffic round-robins with pk
            # and the first matmul starves for ~13us.
            gate_scr = consts.tile([128, 1], bf16)
            first_tiles = []
            prev_gate = pk[:, 0, 0:1]
            for i in range(4):
                t = encp.tile([128, KC, SG2], bf16, tag="enc_tile", name=f"enc_first{i}")
                nc.scalar.copy(t[:, 0, 0:1], prev_gate)
                nc.scalar.copy(t[:, 2, 0:1], prev_gate)
                first_tiles.append(t)
                prev_gate = t[:, 0, 0:1]
            it = 0
            for pr in range(N_PR):
                for b in range(BPC):
                    if it < 4:
                        enc_tile = first_tiles[it]
                    else:
                        enc_tile = encp.tile(
                            [128, KC, SG2], bf16, tag="enc_tile", name="enc_tile"
                        )
                    it += 1
                    nc.sync.dma_start(
                        out=enc_tile[:, 0:2, :], in_=enc_d[b, pr, :, 0:2, :]
                    )
                    nc.sync.dma_start(
                        out=enc_tile[:, 2:4, :], in_=enc_d[b, pr, :, 2:4, :]
                    )
                    # spare last column keeps the claim write disjoint from tanh
                    en_tile = enp.tile([128, DC, SG2 + 1], bf16)
                    # claim the recycled slot: carries the slot-release wait alone
                    nc.scalar.copy(en_tile[:, 0, SG2 : SG2 + 1], bias_act[:, 0:1])
                    scores_a = psp.tile([128, SG], f32, tag="sc", name="sca", bufs=4)
                    scores_b = psp.tile([128, SG], f32, tag="sc", name="scb", bufs=4)
                    for c in range(DC):
                        pp = psp.tile([128, 2, SG], f32, tag="proj", name="pp", bufs=2)
                        for h in range(2):
                            for k in range(KC):
                                nc.tensor.matmul(
                                    pp[:, h, :],
                                    pk[:, k, c * 128 : (c + 1) * 128],
                                    enc_tile[:, k, h * SG : (h + 1) * SG],
                                    start=(k == 0),
                                    stop=(k == KC - 1),
                                )
                        nc.scalar.activation(
                            out=en_tile[:, c, 0:SG2],
                            in_=pp[:, :, :],
                            func=Act.Tanh,
                            bias=bias_act[:, c * BPC + b : c * BPC + b + 1],
                        )
                        nc.tensor.matmul(
                            scores_a[32 * b : 32 * b + 1, :],
                            pk[:, c, _V0 : _V0 + 1],
                            en_tile[:, c, 0:SG],
                            start=(c == 0),
                            stop=(c == DC - 1),
                            tile_position=(0, 32 * b),
                        )
                        nc.tensor.matmul(
                            scores_b[32 * b : 32 * b + 1, :],
                            pk[:, c, _V0 : _V0 + 1],
                            en_tile[:, c, SG:SG2],
                            start=(c == 0),
                            stop=(c == DC - 1),
                            tile_position=(0, 32 * b),
                        )
                    sg = 2 * pr
                    r = slice(32 * b, 32 * b + 1)
                    nc.scalar.activation(
                        out=expd[r, sg * SG : (sg + 1) * SG],
                        in_=scores_a[r, :],
                        func=Act.Exp,
                    )
                    nc.vector.reduce_sum(
                        sums8[r, sg : sg + 1],
                        expd[r, sg * SG : (sg + 1) * SG],
                        axis=mybir.AxisListType.X,
                    )
                    nc.scalar.activation(
                        out=expd[r, (sg + 1) * SG : (sg + 2) * SG],
                        in_=scores_b[r, :],
                        func=Act.Exp,
                    )
                    nc.vector.reduce_sum(
                        sums8[r, sg + 1 : sg + 2],
                        expd[r, (sg + 1) * SG : (sg + 2) * SG],
                        axis=mybir.AxisListType.X,
                    )

            sums = smp.tile([128, 1], f32)
            recip = smp.tile([128, 1], f32)
            out_sb = smp.tile([128, S], f32)
            for b in range(BPC):
                r = slice(32 * b, 32 * b + 1)
                nc.vector.reduce_sum(
                    sums[r, :], sums8[r, :], axis=mybir.AxisListType.X
                )
                nc.vector.reciprocal(recip[r, :], sums[r, :])
                nc.vector.tensor_scalar_mul(
                    out=out_sb[r, :], in0=expd[r, :], scalar1=recip[r, :]
                )
                nc.sync.dma_start(out=out_d[b : b + 1, :], in_=out_sb[r, :])

    _split_multi_waits(nc)
    return nc


def _split_multi_waits(nc):
    """This walrus build allows ONE sync wait per instruction. The kernel body
    is engineered to respect that; Tile's auto-emitted tail drain is not (it
    waits on every processor). Split any multi-wait instruction into a chain
    of single-wait drains on the same engine followed by the original."""
    from concourse import mybir

    for bb in nc.main_func.blocks:
        new_insts = []
        for ins in bb.instructions:
            si = getattr(ins, "sync_info", None)
            if si is not None and si.on_wait and len(si.on_wait) > 1:
                waits = list(si.on_wait)
                for w in waits[:-1]:
                    d = mybir.InstDrain(
                        name=nc.get_next_instruction_name(),
                        ins=[],
                        outs=[],
                        bass_is_fusable=False,
                    )
                    d.engine = ins.engine
                    d.sync_info = mybir.SyncInfo(on_wait=[w], on_update=[])
                    nc.register_instruction(d)
                    new_insts.append(d)
                si.on_wait = waits[-1:]
            new_insts.append(ins)
        bb.instructions[:] = new_insts


def _get_nc():
    global _nc_cache
    if _nc_cache is None:
        _nc_cache = _build_bass()
    return _nc_cache


def _prep_in_maps(decoder_hidden, encoder_outputs, W_attn, b_attn, v):
    decoder_hidden = np.asarray(decoder_hidden, dtype=np.float32)
    encoder_outputs = np.asarray(encoder_outputs, dtype=np.float32)
    W_attn = np.asarray(W_attn, dtype=np.float32)
    b_attn = np.asarray(b_attn, dtype=np.float32)
    v = np.asarray(v, dtype=np.float32)

    W_h = W_attn[:, :DEC]           # [d_out, d_in]
    W_e = W_attn[:, DEC:]           # [d_out, e]

    pk_base = np.zeros((128, KC, NPK), dtype=_BF16)
    pk_base[:, :, _WE0 : _WE0 + DEC] = (
        W_e.T.astype(_BF16).reshape(KC, 128, DEC).transpose(1, 0, 2)
    )
    pk_base[:, :, _WH0 : _WH0 + DEC] = (
        W_h.T.astype(_BF16).reshape(KC, 128, DEC).transpose(1, 0, 2)
    )
    pk_base[:, :, _V0] = v.astype(_BF16).reshape(DC, 128).T
    pk_base[:, :, _B0] = b_attn.astype(_BF16).reshape(DC, 128).T

    # [B, S, E] -> [B, N_PR, 128(p=e%128), KC(e//128), SG2(s)] in bf16
    enc_bt = np.ascontiguousarray(
        encoder_outputs.reshape(B, N_PR, SG2, KC, 128)
        .transpose(0, 1, 4, 3, 2)
        .astype(_BF16)
    )

    in_maps = []
    for core in range(N_CORES):
        sl = slice(core * BPC, (core + 1) * BPC)
        h = decoder_hidden[sl]                                   # [BPC, 512]
        pk = pk_base.copy()
        pk[:, :, _H0 : _H0 + BPC] = (
            h.astype(_BF16).reshape(BPC, KC, 128).transpose(2, 1, 0)
        )
        in_maps.append({"enc_t": enc_bt[sl], "pk": pk})
    return in_maps


def _ensure_ntff_hook():
    """The agent image's ``antenv`` lacks ``axon_hooks``; synthesize it with a
    ctypes-based NTFF profile hook against the injected libaxon (trace runs only)."""
    try:
        from antenv.axon_hooks import get_axon_ntff_profile_hook  # noqa: F401

        return
    except ImportError:
        pass

    import contextlib
    import ctypes
    import types

    so_path = "/opt/axon/libaxon_pjrt.so"
    hook = None
    if os.path.exists(so_path):
        lib = ctypes.CDLL(so_path)
        if hasattr(lib, "axon_start_nrt_profile"):
            lib.axon_start_nrt_profile.argtypes = [
                ctypes.POINTER(ctypes.c_int64),
                ctypes.c_size_t,
            ]
            lib.axon_start_nrt_profile.restype = ctypes.c_int64
            lib.axon_stop_nrt_profile.argtypes = [ctypes.c_char_p]
            lib.axon_stop_nrt_profile.restype = ctypes.c_int64

            @contextlib.contextmanager
            def _hook(output_dir, device_ids):
                import jax

                jax.devices()
                if device_ids:
                    ids = (ctypes.c_int64 * len(device_ids))(*device_ids)
                    rc = lib.axon_start_nrt_profile(ids, len(device_ids))
                else:
                    rc = lib.axon_start_nrt_profile(None, 0)
                if rc != 0:
                    raise RuntimeError(f"axon_start_nrt_profile rc={rc}")
                try:
                    yield
                finally:
                    n = lib.axon_stop_nrt_profile(str(output_dir).encode())
                    if n <= 0:
                        print(f"ntff capture wrote {n} files", file=sys.stderr)

            hook = _hook

    holder = {"h": hook}
    mod = types.ModuleType("antenv.axon_hooks")
    mod.get_axon_ntff_profile_hook = lambda: holder["h"]
    mod.set_axon_ntff_profile_hook = lambda h: holder.__setitem__("h", h)
    sys.modules["antenv.axon_hooks"] = mod
    import antenv

    antenv.axon_hooks = mod


def kernel(decoder_hidden, encoder_outputs, W_attn, b_attn, v):
    global last_results
    import concourse.bass_utils as bass_utils
    from concourse.bass_utils import run_bass_kernel_spmd

    nc = _get_nc()
    in_maps = _prep_in_maps(decoder_hidden, encoder_outputs, W_attn, b_attn, v)

    trace = os.environ.get("BAHDANAU_TRACE", "0") == "1"
    kwargs = {}
    if trace:
        _ensure_ntff_hook()
        bass_utils.upload_artifacts = lambda tmpdir: str(tmpdir)  # no bucket here
        kwargs["trace"] = True
        tmpdir = os.environ.get("BAHDANAU_TRACE_DIR")
        if tmpdir:
            import uuid

            tmpdir = os.path.join(tmpdir, uuid.uuid4().hex[:8])
            os.makedirs(tmpdir, exist_ok=True)
            kwargs["tmpdir"] = tmpdir

    res = run_bass_kernel_spmd(nc, in_maps, core_ids=list(range(N_CORES)), **kwargs)
    last_results = res
    out = np.concatenate([res.results[c]["out"] for c in range(N_CORES)], axis=0)
    return out.astype(np.float32)
